# revision 13
# baseline (speedup 1.0000x reference)
"""GRU-style cell (nn_Lstmcell) on 8 Trainium2 NeuronCores.

h = (1-z)*h_prev + z*tanh((r*h_prev)@whh + x@whx + bh)
r = sigmoid([x,h_prev]@wr + br),  z = sigmoid([x,h_prev]@wz + bz)

Data-parallel over the batch dim: each of the 8 cores gets B/8 rows; the
small weight matrices are replicated. Inputs/weights are fed to the
device as bf16 (host-side cast), halving HBM traffic.

Per-core dataflow — fully feature-major, zero on-chip transposes:
  - x^T, h^T loaded feature-major from DRAM via HWDGE xbar DMA-transpose
    (bf16). The sync ring carries ONLY transposes: mixing plain DMAs
    into the xbar stream costs a multi-us completion handshake per
    class transition, so the single packed weight DMA rides the scalar
    ring and the output is stored once at the end.
  - r^T, z^T, g^T: weights stationary (packed into one SBUF tile),
    activations stream with N=512 into one f32 PSUM bank per matmul.
  - sigmoid/tanh + per-partition bias on ScalarE straight out of PSUM
    (bf16 out); rh and the gated blend on VectorE in bf16 (2x mode).
  - h_out^T accumulates in a resident SBUF buffer; one 2MB store at the
    end. The host transposes back to batch-major (cheap numpy view).
"""

import numpy as np
import ml_dtypes

import concourse.bacc as bacc
import concourse.mybir as mybir
import concourse.tile as tile
from concourse.bass_utils import run_bass_kernel_spmd

NCORES = 8
IN = 256
H = 256
CONCAT = IN + H
CH = 1024  # batch rows per chunk

F32 = mybir.dt.float32
BF16 = mybir.dt.bfloat16
SIG = mybir.ActivationFunctionType.Sigmoid
TANH = mybir.ActivationFunctionType.Tanh

W1_COLS = 8 * H + 16  # wr|wz folded + br|bz|bh bias cols + pad (16-row aligned)
W2_COLS = 4 * H  # whh|whx folded

_BUILD_CACHE = {}
LAST_RESULTS = None


def _build(R):
    """Build + compile the per-core kernel for R batch rows per core."""
    assert R % CH == 0
    n_chunks = R // CH

    nc = bacc.Bacc(
        "TRN2", target_bir_lowering=False, debug=False, num_devices=NCORES
    )

    x_d = nc.dram_tensor("x", [R, IN], BF16, kind="ExternalInput").ap()
    h_d = nc.dram_tensor("h_prev", [R, H], BF16, kind="ExternalInput").ap()
    w1t_d = nc.dram_tensor("w1t", [W1_COLS, 128], BF16, kind="ExternalInput").ap()
    w2t_d = nc.dram_tensor("w2t", [W2_COLS, 128], BF16, kind="ExternalInput").ap()
    out_d = nc.dram_tensor("h_outT", [2 * H // 2, R], BF16, kind="ExternalOutput").ap()

    with tile.TileContext(nc) as tc:
        with (
            tc.tile_pool(name="const", bufs=1) as cpool,
            tc.tile_pool(name="io", bufs=4) as iopool,
            tc.tile_pool(name="work", bufs=2) as wpool,
            tc.tile_pool(name="pr", bufs=2, space="PSUM") as prpool,
            tc.tile_pool(name="pz", bufs=2, space="PSUM") as pzpool,
            tc.tile_pool(name="pg", bufs=3, space="PSUM") as pgpool,
        ):
            w1_sb = cpool.tile([128, W1_COLS], BF16)
            nc.sync.dma_start(w1_sb[:], w1t_d, transpose=True)
            w2_sb = cpool.tile([128, W2_COLS], BF16)
            wr_sb = w1_sb[:, 0 : 4 * H]
            wz_sb = w1_sb[:, 4 * H : 8 * H]
            br_sb = w1_sb[:, 8 * H + 0 : 8 * H + 2]
            bz_sb = w1_sb[:, 8 * H + 2 : 8 * H + 4]
            bh_sb = w1_sb[:, 8 * H + 4 : 8 * H + 6]

            # whole-run h_out^T accumulator: [p, (jc, b)]
            oT_all = cpool.tile([128, 2 * R], BF16)

            for ci in range(n_chunks):
                b0 = ci * CH
                # --- feature-major loads via xbar DMA-transpose (sync ring
                # only carries these) ---
                xT = iopool.tile([128, 2 * CH], BF16, tag="xT")
                hT = iopool.tile([128, 2 * CH], BF16, tag="hT")
                for kc in range(2):
                    nc.sync.dma_start(
                        xT[:, kc * CH : (kc + 1) * CH],
                        x_d[b0 : b0 + CH, kc * 128 : (kc + 1) * 128],
                        transpose=True,
                    )
                    nc.sync.dma_start(
                        hT[:, kc * CH : (kc + 1) * CH],
                        h_d[b0 : b0 + CH, kc * 128 : (kc + 1) * 128],
                        transpose=True,
                    )
                if ci == 0:
                    nc.sync.dma_start(w2_sb[:], w2t_d, transpose=True)

                def xc_sl(kc, lo, n):
                    # feature-major slice of [x; h]^T, chunk kc in 0..3
                    sb = xT if kc < 2 else hT
                    c = kc % 2
                    return sb[:, c * CH + lo : c * CH + lo + n]

                def gate(w_sb, pool, out_sb, func, bias, chunks, korder):
                    # out^T[jc*128+p, b] = func(sum_k w[k,j]*act[k,b] + bias)
                    # kc-major: both half-batch matmuls share one stationary
                    for jc in range(2):
                        ps = [
                            pool.tile([128, 512], F32, tag="ps", name="ps")
                            for _ in range(2)
                        ]
                        for i, kc in enumerate(korder):
                            for hf in range(2):
                                nc.tensor.matmul(
                                    ps[hf][:],
                                    w_sb[
                                        :,
                                        kc * H + jc * 128 : kc * H + jc * 128 + 128,
                                    ],
                                    chunks(kc, hf * 512, 512),
                                    start=(i == 0),
                                    stop=(i == len(korder) - 1),
                                )
                        for hf in range(2):
                            nc.scalar.activation(
                                out_sb[
                                    :, jc * CH + hf * 512 : jc * CH + hf * 512 + 512
                                ],
                                ps[hf][:],
                                func,
                                bias=bias[:, jc : jc + 1],
                            )

                # --- r^T: matmul kc order follows transpose arrival order ---
                r_sb = wpool.tile([128, 2 * CH], BF16, tag="r")
                gate(wr_sb, prpool, r_sb, SIG, br_sb, xc_sl, (0, 2, 1, 3))

                # --- z^T (independent of r; covers the r->rh latency) ---
                z_sb = wpool.tile([128, 2 * CH], BF16, tag="z")
                gate(wz_sb, pzpool, z_sb, SIG, bz_sb, xc_sl, (0, 2, 1, 3))

                # --- rh = r * h^T (feature-major, bf16 2x) ---
                rh = wpool.tile([128, 2 * CH], BF16, tag="rh")
                for jc in range(2):
                    nc.vector.tensor_mul(
                        rh[:, jc * CH : (jc + 1) * CH],
                        r_sb[:, jc * CH : (jc + 1) * CH],
                        hT[:, jc * CH : (jc + 1) * CH],
                    )

                # --- g^T = tanh(rh@whh + x@whx + bh) ---
                def g_sl(kc, lo, n):
                    sb = rh if kc < 2 else xT
                    c = kc % 2
                    return sb[:, c * CH + lo : c * CH + lo + n]

                g_sb = wpool.tile([128, 2 * CH], BF16, tag="g")
                gate(w2_sb[:], pgpool, g_sb, TANH, bh_sb, g_sl, (0, 2, 1, 3))

                # --- blend feature-major: ho = h + z*(g - h), bf16 2x ---
                # (last chunk: 512-wide pieces so the final dependency chain
                # off the last matmul is short)
                d_sb = wpool.tile([128, 2 * CH], BF16, tag="d")
                e_sb = wpool.tile([128, 2 * CH], BF16, tag="e")
                for jc in range(2):
                    sl = slice(jc * CH, (jc + 1) * CH)
                    osl = slice(jc * R + b0, jc * R + b0 + CH)
                    nc.vector.tensor_sub(d_sb[:, sl], g_sb[:, sl], hT[:, sl])
                    nc.vector.tensor_mul(e_sb[:, sl], z_sb[:, sl], d_sb[:, sl])
                    nc.vector.tensor_add(oT_all[:, osl], e_sb[:, sl], hT[:, sl])

            # two half stores at the end (all transposes are long done, so
            # no plain-DMA/transpose handshake); first half overlaps the
            # last chunks' compute
            out_v = out_d.rearrange("(c p) b -> p c b", p=128)
            o_v = oT_all[:].rearrange("p (c b) -> p c b", b=R)
            nc.gpsimd.dma_start(out_v[:, :, 0 : R // 2], o_v[:, :, 0 : R // 2])
            nc.gpsimd.dma_start(
                out_v[:, :, R // 2 : 3 * R // 4], o_v[:, :, R // 2 : 3 * R // 4]
            )
            nc.gpsimd.dma_start(
                out_v[:, :, 3 * R // 4 : 7 * R // 8],
                o_v[:, :, 3 * R // 4 : 7 * R // 8],
            )
            nc.gpsimd.dma_start(
                out_v[:, :, 7 * R // 8 : 15 * R // 16],
                o_v[:, :, 7 * R // 8 : 15 * R // 16],
            )
            nc.gpsimd.dma_start(
                out_v[:, :, 15 * R // 16 : R], o_v[:, :, 15 * R // 16 : R]
            )

    nc.compile()
    return nc


def _bf16(a):
    return np.ascontiguousarray(np.asarray(a, dtype=np.float32)).astype(
        ml_dtypes.bfloat16
    )


def kernel(x, h_prev, wr, wz, whh, whx, br, bz, bh):
    global LAST_RESULTS
    x = _bf16(x).reshape(-1, IN)
    h_prev = _bf16(h_prev).reshape(-1, H)
    B = x.shape[0]
    assert B % NCORES == 0
    R = B // NCORES

    if R not in _BUILD_CACHE:
        _BUILD_CACHE[R] = _build(R)
    nc = _BUILD_CACHE[R]

    def _fold(w, nchunk):
        w = _bf16(w)
        return w.reshape(nchunk, 128, H).transpose(1, 0, 2).reshape(128, nchunk * H)

    def _bias_fold(b):
        # [H] -> per-partition [128, 2] feature-major (jc chunks)
        return _bf16(b).reshape(2, 128).T

    w1 = np.zeros((128, W1_COLS), dtype=ml_dtypes.bfloat16)
    w1[:, 0 : 4 * H] = _fold(wr, 4)
    w1[:, 4 * H : 8 * H] = _fold(wz, 4)
    w1[:, 8 * H + 0 : 8 * H + 2] = _bias_fold(br)
    w1[:, 8 * H + 2 : 8 * H + 4] = _bias_fold(bz)
    w1[:, 8 * H + 4 : 8 * H + 6] = _bias_fold(bh)
    w2 = np.concatenate([_fold(whh, 2), _fold(whx, 2)], axis=1)
    w1t = np.ascontiguousarray(w1.T)
    w2t = np.ascontiguousarray(w2.T)

    in_maps = []
    for i in range(NCORES):
        in_maps.append(
            {
                "w1t": w1t,
                "w2t": w2t,
                "x": x[i * R : (i + 1) * R],
                "h_prev": h_prev[i * R : (i + 1) * R],
            }
        )

    res = run_bass_kernel_spmd(nc, in_maps, list(range(NCORES)))
    LAST_RESULTS = res
    # h_outT is [256, R] feature-major; transpose back on the host
    out = np.concatenate(
        [
            np.asarray(res.results[i]["h_outT"], dtype=np.float32).T
            for i in range(NCORES)
        ],
        axis=0,
    )
    return np.ascontiguousarray(out).reshape(B, 1, H)
